# revision 13
# baseline (speedup 1.0000x reference)
"""GAT-style message passing (nn_CRMF_8366596293120) on 8 Trainium2 cores.

Math (per batch b, head h):
  h' = h @ w[h]                     [n, 64]
  src_i = h' @ wa_src ; dst_j = h' @ wa_dst        (wa = w[h] @ a)
  x[i,j] = src_i + dst_j
  attn_u = exp(leaky_relu(x, 0.2))  = max(A_i*u_j, B_i*v_j)
     with A=e^src, B=e^{0.2 src}, u=e^dst, v=e^{0.2 dst}   (rank-1 pieces!)
  attn = attn_u / rowsum_i ;  out = attn @ (h'+b)

Per-core plan (core c handles batch c; params replicated):
  T-layout [j,i]:  S_T = u_j A_i - v_j B_i  (K=2 outer product on PE)
                   R_T = relu(S_T)          (DVE tensor_scalar max-0 exit)
                   O_u.T = Hext.T @ R_T  (+ rank-1 fixup (Hext.T v) x B_row)
                   Hext = [h'+b | ones] -> ones column gives rowsum for free.
  [i,j]-layout:    x = src_i + dst_j        (K=2 outer product on PE)
                   L = Lrelu(x)             (ACT)
                   attn = Exp(L - ln(rowsum_i))   (ACT, per-partition bias
                                                   folds the softmax divide)
  O = transpose(O_u.T) * recip_i  (PE transpose blocks + DVE scale exit)
"""

import numpy as np

N = 2048
FIN = 32
FOUT = 64
FEXT = FOUT + 1  # h' columns + ones column
NHEAD = 4
P = 128
NT = N // P  # 16 row tiles
NCORES = 8
BS = 8

# fraction of T-side relu exits routed to ACT instead of DVE (engine balance)
RELU_ACT_TILES = 0  # 0..NT; tune from profile


def build_nc():
    from contextlib import ExitStack

    import concourse.bass as bass  # noqa: F401
    import concourse.mybir as mybir
    import concourse.tile as tile
    from concourse import bacc
    from concourse.masks import make_identity

    dt = mybir.dt
    f32 = dt.float32
    AF = mybir.ActivationFunctionType
    OP = mybir.AluOpType

    nc = bacc.Bacc(None, target_bir_lowering=False)

    h_in = nc.dram_tensor("h", [N, FIN], f32, kind="ExternalInput")
    w_in = nc.dram_tensor("w", [NHEAD, FIN, FOUT], f32, kind="ExternalInput")
    asrc_in = nc.dram_tensor("a_src", [NHEAD, FOUT, 1], f32, kind="ExternalInput")
    adst_in = nc.dram_tensor("a_dst", [NHEAD, FOUT, 1], f32, kind="ExternalInput")
    b_in = nc.dram_tensor("b", [FOUT], f32, kind="ExternalInput")
    out_o = nc.dram_tensor("out", [NHEAD, N, FOUT], f32, kind="ExternalOutput")
    rs_dram = nc.dram_tensor("rs_scratch", [NHEAD, N], f32, kind="Internal")
    out_attn = nc.dram_tensor("attn", [NHEAD, N, N], f32, kind="ExternalOutput")

    with tile.TileContext(nc) as tc, ExitStack() as ctx:
        const = ctx.enter_context(tc.tile_pool(name="const", bufs=1))
        vec = ctx.enter_context(tc.tile_pool(name="vec", bufs=2))
        rpool = ctx.enter_context(tc.tile_pool(name="rT", bufs=3))
        lpool = ctx.enter_context(tc.tile_pool(name="lrelu", bufs=2))
        apool = ctx.enter_context(tc.tile_pool(name="attnp", bufs=3))
        hxpool = ctx.enter_context(tc.tile_pool(name="hext", bufs=3))
        otpool = ctx.enter_context(tc.tile_pool(name="otsb", bufs=1))
        opool = ctx.enter_context(tc.tile_pool(name="osb", bufs=3))
        mmps = ctx.enter_context(tc.tile_pool(name="mmps", bufs=2, space="PSUM"))
        otps = ctx.enter_context(tc.tile_pool(name="otps", bufs=1, space="PSUM"))

        # ---------------- setup ----------------
        identity = const.tile([P, P], f32)
        make_identity(nc, identity)

        # h tiles: hsb[:, t, :] = h[t*128:(t+1)*128, :]
        hsb = const.tile([P, NT, FIN], f32)
        nc.sync.dma_start(out=hsb, in_=h_in.rearrange("(t p) k -> p t k", p=P))

        # hT [32, 2048] = h.T  via PE transposes
        hT = const.tile([FIN, N], f32)
        for t in range(NT):
            pt = mmps.tile([FIN, P], f32, tag="mm")
            nc.tensor.transpose(pt, hsb[:, t, :], identity)
            nc.scalar.activation(hT[:, t * P:(t + 1) * P], pt, AF.Copy)

        # w (natural [fin, fout] per head) and wT per head
        w_sb = const.tile([FIN, NHEAD, FOUT], f32)
        nc.sync.dma_start(out=w_sb, in_=w_in.rearrange("h k f -> k h f"))
        wT = const.tile([FOUT, NHEAD * FIN], f32)
        for hd in range(NHEAD):
            pt = mmps.tile([FOUT, FIN], f32, tag="mm")
            nc.tensor.transpose(
                pt, w_sb[:, hd, :], identity[0:FIN, 0:FIN]
            )
            nc.scalar.activation(wT[:, hd * FIN:(hd + 1) * FIN], pt, AF.Copy)

        # a_src / a_dst columns [64, nhead each]
        aall = const.tile([FOUT, 2, NHEAD], f32)
        nc.sync.dma_start(
            out=aall[:, 0, :], in_=asrc_in.rearrange("h f o -> f (o h)")
        )
        nc.sync.dma_start(
            out=aall[:, 1, :], in_=adst_in.rearrange("h f o -> f (o h)")
        )

        # b replicated across partitions
        ones1p = const.tile([1, P], f32)
        nc.vector.memset(ones1p, 1.0)
        brow = const.tile([1, FOUT], f32)
        nc.sync.dma_start(out=brow, in_=b_in.rearrange("(o f) -> o f", o=1))
        brep_ps = mmps.tile([P, FOUT], f32, tag="mm")
        nc.tensor.matmul(brep_ps, ones1p, brow, start=True, stop=True)
        brep = const.tile([P, FOUT], f32)
        nc.scalar.activation(brep, brep_ps, AF.Copy)

        # ---------------- per head ----------------
        for hd in range(NHEAD):
            wslice = w_sb[:, hd, :]
            wTslice = wT[:, hd * FIN:(hd + 1) * FIN]

            # wa columns: [wa_src, 0.2*wa_src, wa_dst, 0.2*wa_dst]
            wa_ps = mmps.tile([FIN, 2], f32, tag="mm")
            nc.tensor.matmul(
                wa_ps[:, 0:1], wTslice, aall[:, 0, hd:hd + 1], start=True, stop=True
            )
            nc.tensor.matmul(
                wa_ps[:, 1:2], wTslice, aall[:, 1, hd:hd + 1],
                start=True, stop=True,
            )
            wa = vec.tile([FIN, 4], f32, tag="wa")
            nc.vector.tensor_copy(wa[:, 0:4:2], wa_ps)  # cols 0,2 = src,dst
            nc.vector.tensor_scalar_mul(wa[:, 1:4:2], wa_ps, 0.2)  # cols 1,3

            # vector rows (all at base partition 0)
            urow = vec.tile([1, N], f32, tag="urow")      # u = e^dst
            arow = vec.tile([1, N], f32, tag="arow")      # A = e^src
            brow_h = vec.tile([1, N], f32, tag="brow_h")  # B = e^{0.2 src}
            dst_tmp = vec.tile([1, N], f32, tag="dst_tmp")
            lhsT_ij = vec.tile([2, N], f32, tag="lhsT_ij")  # [src; ones]
            rhs_ij = vec.tile([2, N], f32, tag="rhs_ij")    # [ones; dst]
            # ones rows: memset whole 2-partition tiles, then overwrite row 0
            # (engines cannot address base partition 1 directly)
            nc.vector.memset(lhsT_ij, 1.0)
            nc.vector.memset(rhs_ij, 1.0)

            HALF = N // 2
            for half in range(2):
                hs = slice(half * HALF, (half + 1) * HALF)
                # src row
                sda = mmps.tile([1, HALF], f32, tag="mm")
                for k in range(HALF // 512):
                    off = half * HALF + k * 512
                    nc.tensor.matmul(
                        sda[:, k * 512:(k + 1) * 512],
                        wa[:, 0:1], hT[:, off:off + 512],
                        start=True, stop=True,
                    )
                nc.scalar.activation(arow[:, hs], sda, AF.Exp)
                nc.scalar.activation(brow_h[:, hs], sda, AF.Exp, scale=0.2)
                nc.scalar.activation(lhsT_ij[0:1, hs], sda, AF.Copy)
                # dst row
                sdb = mmps.tile([1, HALF], f32, tag="mm")
                for k in range(HALF // 512):
                    off = half * HALF + k * 512
                    nc.tensor.matmul(
                        sdb[:, k * 512:(k + 1) * 512],
                        wa[:, 2:3], hT[:, off:off + 512],
                        start=True, stop=True,
                    )
                nc.scalar.activation(urow[:, hs], sdb, AF.Exp)
                nc.scalar.activation(dst_tmp[:, hs], sdb, AF.Copy)
            # dst row -> partition 1 of rhs_ij (partition move => DMA)
            nc.sync.dma_start(out=rhs_ij[1:2, :], in_=dst_tmp)

            # v_col [128, 16]: v = e^{0.2 dst} in column form
            dc_ps = mmps.tile([P, NT], f32, tag="mm")
            for t in range(NT):
                nc.tensor.matmul(
                    dc_ps[:, t:t + 1],
                    hT[:, t * P:(t + 1) * P], wa[:, 3:4],
                    start=True, stop=True,
                )
            v_col = vec.tile([P, NT], f32, tag="v_col")
            nc.scalar.activation(v_col, dc_ps, AF.Exp)

            # B replicated across partitions: B_rep = ones (x) B_row
            brep_h_ps0 = mmps.tile([P, HALF], f32, tag="mm")
            brep_h_ps1 = mmps.tile([P, HALF], f32, tag="mm")
            b_rep_h = const.tile([P, N], f32, tag=f"b_rep_h{hd % 2}")
            for half, bps in ((0, brep_h_ps0), (1, brep_h_ps1)):
                for k in range(HALF // 512):
                    off = half * HALF + k * 512
                    nc.tensor.matmul(
                        bps[:, k * 512:(k + 1) * 512],
                        ones1p, brow_h[:, off:off + 512],
                        start=True, stop=True,
                    )
                hs = slice(half * HALF, (half + 1) * HALF)
                nc.scalar.activation(b_rep_h[:, hs], bps, AF.Copy)

            # ---------------- T phase ----------------
            # attn_uT[j, i] = max(u_j * A_i, v_j * B_i), O_u.T = Hext.T @ attn_uT
            ot_ps = otps.tile([FEXT, N], f32, tag="ot")
            for jt in range(NT):
                jl = slice(jt * P, (jt + 1) * P)
                # h' tile and Hext = [h'+b | 1]
                hp_ps = mmps.tile([P, FOUT], f32, tag="mm")
                nc.tensor.matmul(hp_ps, hT[:, jl], wslice, start=True, stop=True)
                hext = hxpool.tile([P, FEXT], f32, tag="hext")
                nc.vector.tensor_tensor(
                    hext[:, 0:FOUT], hp_ps, brep, op=OP.add
                )
                nc.vector.memset(hext[:, FOUT:FEXT], 1.0)

                auT = rpool.tile([P, N], f32, tag="rT")
                for half in range(2):
                    st_ps = mmps.tile([P, HALF], f32, tag="mm")
                    for k in range(HALF // 512):
                        off = half * HALF + k * 512
                        nc.tensor.matmul(
                            st_ps[:, k * 512:(k + 1) * 512],
                            urow[:, jl], arow[:, off:off + 512],
                            start=True, stop=True,
                        )
                    hs = slice(half * HALF, (half + 1) * HALF)
                    # max(e1T, v_j * B_i) in one pass
                    nc.vector.scalar_tensor_tensor(
                        auT[:, hs], b_rep_h[:, hs], v_col[:, jt:jt + 1],
                        st_ps, op0=OP.mult, op1=OP.max,
                    )

                # O_u.T accumulation
                for k in range(N // 512):
                    nc.tensor.matmul(
                        ot_ps[0:FEXT, k * 512:(k + 1) * 512],
                        hext, auT[:, k * 512:(k + 1) * 512],
                        start=(jt == 0), stop=(jt == NT - 1),
                    )

            # exit O_u.T, rowsum -> recip / -ln
            ot_sb = otpool.tile([FEXT, N], f32, tag="otsb")
            nc.vector.tensor_copy(ot_sb, ot_ps[0:FEXT, :])
            rowsum_col = vec.tile([P, NT], f32, tag="rowsum_col")
            nc.sync.dma_start(out=rs_dram[hd, :], in_=ot_sb[FOUT:FEXT, :])
            nc.sync.dma_start(
                out=rowsum_col, in_=rs_dram[hd, :].rearrange("(t p) -> p t", p=P)
            )
            recip_col = vec.tile([P, NT], f32, tag="recip_col")
            nc.vector.reciprocal(recip_col, rowsum_col)
            negln_col = vec.tile([P, NT], f32, tag="negln_col")
            nc.scalar.activation(negln_col, recip_col, AF.Ln)

            # O = transpose blocks * recip
            for it in range(NT):
                il = slice(it * P, (it + 1) * P)
                tr_ps = mmps.tile([P, FEXT], f32, tag="mm")
                nc.tensor.transpose(
                    tr_ps, ot_sb[:, il], identity[0:FEXT, 0:FEXT]
                )
                o_sb = opool.tile([P, FOUT], f32, tag="osb")
                nc.vector.tensor_scalar(
                    o_sb, tr_ps[:, 0:FOUT], recip_col[:, it:it + 1], None,
                    op0=OP.mult,
                )
                nc.sync.dma_start(out=out_o[hd, il, :], in_=o_sb)

            # ---------------- [i,j] phase ----------------
            for it in range(NT):
                il = slice(it * P, (it + 1) * P)
                l_sb = lpool.tile([P, N], f32, tag="lrelu")
                for half in range(2):
                    x_ps = mmps.tile([P, HALF], f32, tag="mm")
                    for k in range(HALF // 512):
                        off = half * HALF + k * 512
                        nc.tensor.matmul(
                            x_ps[:, k * 512:(k + 1) * 512],
                            lhsT_ij[:, il], rhs_ij[:, off:off + 512],
                            start=True, stop=True,
                        )
                    hs = slice(half * HALF, (half + 1) * HALF)
                    nc.scalar.activation(l_sb[:, hs], x_ps, AF.Prelu, alpha=0.2)
                attn_sb = apool.tile([P, N], f32, tag="attnp")
                nc.scalar.activation(
                    attn_sb, l_sb, AF.Exp, bias=negln_col[:, it:it + 1]
                )
                nc.sync.dma_start(out=out_attn[hd, il, :], in_=attn_sb)

    nc.compile()
    return nc


def kernel(h, w, a_src, a_dst, b):
    from concourse.bass_utils import run_bass_kernel_spmd

    nc = build_nc()
    in_maps = []
    for c in range(NCORES):
        in_maps.append(
            {
                "h": np.ascontiguousarray(h[c]).astype(np.float32),
                "w": np.ascontiguousarray(w).astype(np.float32),
                "a_src": np.ascontiguousarray(a_src).astype(np.float32),
                "a_dst": np.ascontiguousarray(a_dst).astype(np.float32),
                "b": np.ascontiguousarray(b).astype(np.float32),
            }
        )
    import os
    trace = bool(os.environ.get("BASS_TRACE"))
    res = run_bass_kernel_spmd(
        nc, in_maps, core_ids=list(range(NCORES)), trace=trace
    )
    global _last_results
    _last_results = res
    out = np.stack([r["out"] for r in res.results])
    attn = np.stack([r["attn"] for r in res.results])
    return out, attn


_last_results = None
